# revision 14
# baseline (speedup 1.0000x reference)
"""Trainium2 Bass kernel for nn_Attention_32220844654630 (GNN message passing).

Self-contained: kernel(**inputs) -> 5-tuple matching reference outputs.

Strategy: partition voxels across 8 cores (edges are sorted by destination
voxel, so each core gets a contiguous edge range; no collectives needed).
Per core, segments (per-voxel edge groups) are sorted by length and packed
into 49 blocks of 128 segments; each block is padded to its max segment
length L_b, giving a slot grid [128 seg-partitions x NCOL columns] with only
~4% padding.  Per-edge math runs in H-on-partitions layout:
  Z = Wv.T@X[v] (block-broadcast matmul) + Wp.T@P[p] (matmul over host-staged
      gathered raw P columns) accumulated in PSUM,
  T = tanh(Z + bv + bp) on ACT straight from PSUM,
  att = theta.T @ T per 128-slot column on PE.
Segment softmax stats are uniform blocked reduces on DVE; the weighted
scatter-sum uses host-staged P rows (bf16), a broadcast-multiply by soft_att,
and identity-matmul PSUM accumulation per block.  Host only reshards /
gathers / transposes raw inputs; all arithmetic is on device.
"""
import sys
import types
import numpy as np

N_CORES = 8
H = 128
NP_N = 1000
NV = 50000
NVS = NV // N_CORES        # 6250
NVS_PAD = 6272             # 49 * 128
NBLK = NVS_PAD // 128      # 49
GATT_PAD = np.float32(-60000.0)


# ---------------------------------------------------------------- axon hooks
def _setup_env():
    try:
        import antenv
    except ImportError:
        return
    if "antenv.axon_hooks" not in sys.modules:
        hooks = types.ModuleType("antenv.axon_hooks")
        _h = [None]
        hooks.set_axon_ntff_profile_hook = lambda h: _h.__setitem__(0, h)
        hooks.get_axon_ntff_profile_hook = lambda: _h[0]
        sys.modules["antenv.axon_hooks"] = hooks
        antenv.axon_hooks = hooks
        try:
            from trn_agent_boot.trn_boot import _ntff_profile_via_ctypes
            hook = _ntff_profile_via_ctypes("/opt/axon/libaxon_pjrt.so")
            if hook is not None:
                hooks.set_axon_ntff_profile_hook(hook)
        except Exception:
            pass
    import concourse.bass_utils as bu
    bu.upload_artifacts = lambda tmpdir: f"file://{tmpdir}"


# ---------------------------------------------------------------- host prep
def _core_layout(vidx, e0, e1, v0, Ls_force=None):
    vloc = (vidx[e0:e1] - v0).astype(np.int64)
    counts = np.bincount(vloc, minlength=NVS_PAD)
    offs = np.concatenate([[0], np.cumsum(counts)])
    order = np.argsort(counts, kind="stable")
    Ls = np.zeros(NBLK, dtype=np.int64)
    for b in range(NBLK):
        Ls[b] = counts[order[b * 128 : (b + 1) * 128]].max()
    if Ls_force is not None:
        assert (Ls_force >= Ls).all()
        Ls = Ls_force.copy()
    colstart = np.concatenate([[0], np.cumsum(Ls)])
    ncol = int(Ls.sum())
    slot_edge = np.full((128, ncol), -1, dtype=np.int64)
    for b in range(NBLK):
        segs = order[b * 128 : (b + 1) * 128]
        c0 = int(colstart[b])
        for p in range(128):
            ln = int(counts[segs[p]])
            if ln:
                slot_edge[p, c0 : c0 + ln] = e0 + offs[segs[p]] + np.arange(ln)
    return dict(Ec=e1 - e0, ncol=ncol, Ls=Ls, colstart=colstart, order=order,
                counts=counts, slot_edge=slot_edge)


def _prep(inputs):
    pidx = np.asarray(inputs["cross_edge_program_index"]).astype(np.int64)
    vidx = np.asarray(inputs["cross_edge_voxel_index"]).astype(np.int64)
    X = np.asarray(inputs["voxel_feature"], dtype=np.float32)
    P = np.asarray(inputs["program_graph_feature"], dtype=np.float32)
    g_att = np.asarray(inputs["g_att"], dtype=np.float32)
    g_mask = np.asarray(inputs["g_mask"], dtype=np.float32)

    bounds = [int(np.searchsorted(vidx, c * NVS, side="left"))
              for c in range(N_CORES)] + [len(vidx)]
    lays = [_core_layout(vidx, bounds[c], bounds[c + 1], c * NVS)
            for c in range(N_CORES)]
    # SPMD: one program for all cores -> common per-block lengths
    Ls_common = np.max([l["Ls"] for l in lays], axis=0)
    lays = [_core_layout(vidx, bounds[c], bounds[c + 1], c * NVS,
                         Ls_force=Ls_common)
            for c in range(N_CORES)]
    ncol = int(Ls_common.sum())

    itile = np.tile(np.eye(128, dtype=np.float32), (1, 4))          # [128,512]
    i128bf = np.eye(128, dtype=np.float32)

    in_maps = []
    for c, lay in enumerate(lays):
        v0 = c * NVS
        order = lay["order"]
        se = lay["slot_edge"]
        pad = se < 0
        e_ids = np.where(pad, 0, se)
        p_slot = np.where(pad, 0, pidx[e_ids]).astype(np.int64)      # [128,ncol]
        gatt = np.where(pad, GATT_PAD, g_att[e_ids].astype(np.float32))

        Xs = np.zeros((NVS_PAD, H), dtype=np.float32)
        Xs[:NVS] = X[v0 : v0 + NVS]
        Xseg = Xs[order]                                             # seg-order rows
        gmp = np.zeros((NVS_PAD, 2), dtype=np.float32)
        gmp[:NVS] = g_mask[v0 : v0 + NVS]
        gms = gmp[order]                                             # seg-order

        # P gathered per slot. prawT: [128 H, nslot] fp16, slot j = col*128+p
        Pg = P[p_slot]                                               # [128,ncol,128]
        prawT = np.ascontiguousarray(
            Pg.transpose(2, 1, 0).reshape(H, ncol * 128)).astype(np.float16)
        praw_rows = np.ascontiguousarray(
            Pg.reshape(128, ncol * 128))
        import ml_dtypes
        praw_rows = praw_rows.astype(ml_dtypes.bfloat16)

        in_maps.append(dict(
            xt=np.ascontiguousarray(Xseg.T),                         # [128,6272]
            xrows=np.ascontiguousarray(Xseg),                        # [6272,128]
            prawT=prawT,                                             # [128,nslot]
            praw_rows=praw_rows,                                     # [128,nslot]
            gatt=np.ascontiguousarray(gatt),                         # [128,ncol]
            gm0=np.ascontiguousarray(gms[:, 0].reshape(NBLK, 128).T),  # [128,49]
            gm1=np.ascontiguousarray(gms[:, 1].reshape(NBLK, 128).T),
            Wv=np.asarray(inputs["Wv"], np.float32),
            Wp=np.asarray(inputs["Wp"], np.float32),
            W1=np.asarray(inputs["W1"], np.float32),
            W2=np.asarray(inputs["W2"], np.float32),                 # [64,2]
            b2r=np.asarray(inputs["b2"], np.float32).reshape(1, 2),
            b1c=np.asarray(inputs["b1"], np.float32).reshape(64, 1),
            bvc=np.asarray(inputs["bv"], np.float32).reshape(128, 1),
            bpc=np.asarray(inputs["bp"], np.float32).reshape(128, 1),
            theta=np.asarray(inputs["theta"], np.float32).reshape(128, 1),
            itile=itile,
            i128bf=i128bf.astype(ml_dtypes.bfloat16),
        ))
    return in_maps, dict(lays=lays, bounds=bounds, ncol=ncol)


# ---------------------------------------------------------------- device
def _build(ncol, Ls, colstart):
    LMAX = int(max(Ls))
    import concourse.bass as bass
    import concourse.bacc as bacc
    import concourse.mybir as mybir
    from concourse.tile import TileContext

    dt = mybir.dt
    F32, F16, BF16 = dt.float32, dt.float16, dt.bfloat16
    AF = mybir.ActivationFunctionType
    OP = mybir.AluOpType
    nslot = ncol * 128

    nc = bacc.Bacc(None, target_bir_lowering=False, debug=False)
    P_ = {}
    def dram(name, shape, dty, out=False):
        P_[name] = nc.declare_dram_parameter(name, shape, dty, isOutput=out)
        return P_[name]

    xt = dram("xt", [128, NVS_PAD], F32)
    xrows = dram("xrows", [NVS_PAD, H], F32)
    prawT = dram("prawT", [128, nslot], F16)
    praw_rows = dram("praw_rows", [128, nslot], BF16)
    gatt = dram("gatt", [128, ncol], F32)
    gm0 = dram("gm0", [128, NBLK], F32)
    gm1 = dram("gm1", [128, NBLK], F32)
    Wv = dram("Wv", [128, 128], F32)
    Wp = dram("Wp", [128, 128], F32)
    W1 = dram("W1", [128, 64], F32)
    W2 = dram("W2", [64, 2], F32)
    b2r = dram("b2r", [1, 2], F32)
    b1c = dram("b1c", [64, 1], F32)
    bvc = dram("bvc", [128, 1], F32)
    bpc = dram("bpc", [128, 1], F32)
    theta = dram("theta", [128, 1], F32)
    itile = dram("itile", [128, 512], F32)
    i128bf = dram("i128bf", [128, 128], BF16)

    soft_g = dram("soft_g", [128, ncol], F32, out=True)
    hard_g = dram("hard_g", [128, ncol], F32, out=True)
    s0_pk = dram("s0_pk", [128, NBLK], F32, out=True)
    hm_pk = dram("hm_pk", [128, NBLK], F32, out=True)
    newr = dram("newr", [NVS_PAD, H], F32, out=True)

    ddram = nc.dram_tensor("ddram", [1, NVS_PAD], F32)

    # distinct-L groups (consecutive blocks with equal L)
    groups = []
    b = 0
    while b < NBLK:
        e = b
        while e < NBLK and Ls[e] == Ls[b]:
            e += 1
        groups.append((b, e, int(Ls[b])))
        b = e

    with TileContext(nc) as tc:
        with (
            tc.tile_pool(name="const", bufs=1) as cst,
            tc.tile_pool(name="tab", bufs=1) as tab,
            tc.tile_pool(name="st", bufs=1) as st,
            tc.tile_pool(name="wk", bufs=2) as wk,
            tc.tile_pool(name="wk3", bufs=3) as wk3,
            tc.tile_pool(name="blk", bufs=2) as blk,
            tc.tile_pool(name="ps", bufs=1, space="PSUM") as ps,
            tc.tile_pool(name="psz", bufs=3, space="PSUM") as psz,
            tc.tile_pool(name="psa", bufs=2, space="PSUM") as psa,
        ):
            # ---- load constants / tables
            t_xt = tab.tile([128, NVS_PAD], F32, tag="xt")
            nc.sync.dma_start(t_xt[:], xt.ap())
            t_Wv = cst.tile([128, 128], F32, tag="wv"); nc.sync.dma_start(t_Wv[:], Wv.ap())
            t_Wp = cst.tile([128, 128], F32, tag="wp"); nc.sync.dma_start(t_Wp[:], Wp.ap())
            t_W1 = cst.tile([128, 64], F32, tag="w1"); nc.sync.dma_start(t_W1[:], W1.ap())
            t_W2 = cst.tile([64, 2], F32, tag="w2"); nc.sync.dma_start(t_W2[:], W2.ap())
            t_b2r = cst.tile([1, 2], F32, tag="b2r"); nc.sync.dma_start(t_b2r[:], b2r.ap())
            t_b1c = cst.tile([64, 1], F32, tag="b1c"); nc.sync.dma_start(t_b1c[:], b1c.ap())
            t_bvc = cst.tile([128, 1], F32, tag="bvc"); nc.sync.dma_start(t_bvc[:], bvc.ap())
            t_bpc = cst.tile([128, 1], F32, tag="bpc"); nc.sync.dma_start(t_bpc[:], bpc.ap())
            t_th = cst.tile([128, 1], F32, tag="th"); nc.sync.dma_start(t_th[:], theta.ap())
            t_ibf = cst.tile([128, 128], BF16, tag="ibf"); nc.sync.dma_start(t_ibf[:], i128bf.ap())
            t_gatt = tab.tile([128, ncol], F32, tag="gatt")
            nc.sync.dma_start(t_gatt[:], gatt.ap())
            t_gm0 = cst.tile([128, NBLK], F32, tag="gm0"); nc.sync.dma_start(t_gm0[:], gm0.ap())
            t_gm1 = cst.tile([128, NBLK], F32, tag="gm1"); nc.sync.dma_start(t_gm1[:], gm1.ap())

            t_Wp16 = cst.tile([128, 128], F16, tag="wp16")
            nc.vector.tensor_copy(t_Wp16[:], t_Wp[:])
            t_bvbp = cst.tile([128, 1], F32, tag="bvbp")
            nc.vector.tensor_tensor(t_bvbp[:], t_bvc[:], t_bpc[:], op=OP.add)
            t_W2d = cst.tile([64, 1], F32, tag="w2d")
            nc.vector.tensor_tensor(t_W2d[:], t_W2[:, 1:2], t_W2[:, 0:1], op=OP.subtract)
            t_b2d = cst.tile([1, 1], F32, tag="b2d")
            nc.vector.tensor_tensor(t_b2d[:], t_b2r[:, 1:2], t_b2r[:, 0:1], op=OP.subtract)

            # ---- XvT = Wv.T @ xt  -> [128 H, 6272 seg]
            t_xv = tab.tile([128, NVS_PAD], F32, tag="xvt")
            for n in range(0, NVS_PAD, 512):
                w0 = min(512, NVS_PAD - n)
                p0 = ps.tile([128, 512], F32, tag="ps_small")
                nc.tensor.matmul(p0[:, :w0], t_Wv[:], t_xt[:, n : n + w0],
                                 start=True, stop=True)
                nc.vector.tensor_copy(t_xv[:, n : n + w0], p0[:, :w0])

            # ---- mask MLP: h1T = W1.T @ xt + b1 ; d = W2d.T @ h1T + b2d
            t_h1 = tab.tile([64, NVS_PAD], F32, tag="h1t")
            for n in range(0, NVS_PAD, 512):
                w = min(512, NVS_PAD - n)
                p0 = ps.tile([128, 512], F32, tag="ps_small")
                nc.tensor.matmul(p0[:64, :w], t_W1[:], t_xt[:, n : n + w],
                                 start=True, stop=True)
                nc.vector.tensor_scalar(t_h1[:, n : n + w], p0[:64, :w],
                                        t_b1c[:, 0:1], None, op0=OP.add)
            t_d = st.tile([1, NVS_PAD], F32, tag="dtile")
            for n in range(0, NVS_PAD, 512):
                w = min(512, NVS_PAD - n)
                p0 = ps.tile([128, 512], F32, tag="ps_small")
                nc.tensor.matmul(p0[:1, :w], t_W2d[:], t_h1[:, n : n + w],
                                 start=True, stop=True)
                nc.vector.tensor_scalar(t_d[:, n : n + w], p0[:1, :w],
                                        t_b2d[:, 0:1], None, op0=OP.add)
            nc.sync.dma_start(ddram[:], t_d[:])
            t_dpk = st.tile([128, NBLK], F32, tag="dpk")
            nc.sync.dma_start(
                t_dpk[:], ddram.ap().rearrange("a (b p) -> p (a b)", p=128))
            t_dg = st.tile([128, NBLK], F32, tag="dg")
            nc.vector.tensor_tensor(t_dg[:], t_dpk[:], t_gm1[:], op=OP.add)
            nc.vector.tensor_tensor(t_dg[:], t_dg[:], t_gm0[:], op=OP.subtract)
            t_s0 = st.tile([128, NBLK], F32, tag="s0")
            nc.scalar.activation(t_s0[:], t_dg[:], AF.Sigmoid, scale=-1.0)
            t_h0 = st.tile([128, NBLK], F32, tag="h0")
            nc.vector.tensor_scalar(t_h0[:], t_dg[:], 0.0, None, op0=OP.is_le)
            t_hm = st.tile([128, NBLK], F32, tag="hm")
            nc.vector.tensor_tensor(t_hm[:], t_h0[:], t_s0[:], op=OP.subtract)
            nc.vector.tensor_tensor(t_hm[:], t_hm[:], t_s0[:], op=OP.add)
            nc.sync.dma_start(s0_pk.ap(), t_s0[:])
            nc.sync.dma_start(hm_pk.ap(), t_hm[:])

            # ---- pipelined per-L-group: att -> stats -> soft/hard -> agg
            for (g0, g1, L) in groups:
                if L == 0:
                    for b in range(g0, g1):
                        t_xr = blk.tile([128, 128], F32, tag="xrblk")
                        nc.sync.dma_start(t_xr[:],
                                          xrows.ap()[b * 128 : (b + 1) * 128, :])
                        t_new = blk.tile([128, 128], F32, tag="newblk")
                        nc.vector.tensor_copy(t_new[:], t_xr[:])
                        nc.sync.dma_start(newr.ap()[b * 128 : (b + 1) * 128, :],
                                          t_new[:])
                    continue
                nb = g1 - g0
                gc0 = int(colstart[g0])
                gcols = nb * L
                # --- att for this group
                t_att = wk.tile([128, 640], F32, tag="attg")
                done = 0
                while done < gcols:
                    cw = min(512, gcols - done)
                    att_ps = psa.tile([128, 512], F32, tag="attps")
                    filled = 0
                    while filled < cw:
                        b = g0 + (done + filled) // L
                        l = (done + filled) % L
                        w = min(4, L - l, cw - filled)
                        c0 = int(colstart[b])
                        if l == 0:
                            t_pr = wk.tile([128, LMAX * 128], F16, tag="prbuf")
                            nc.sync.dma_start(
                                t_pr[:, : L * 128],
                                prawT.ap()[:, c0 * 128 : (c0 + L) * 128])
                        zp = psz.tile([128, 512], F32, tag="zps")
                        nc.tensor.matmul(zp[:, : w * 128], t_Wp16[:],
                                         t_pr[:, l * 128 : (l + w) * 128],
                                         start=True, stop=True)
                        t_Z = wk.tile([128, 512], F32, tag="ztile")
                        for j in range(w):
                            nc.vector.tensor_tensor(
                                t_Z[:, j * 128 : (j + 1) * 128],
                                zp[:, j * 128 : (j + 1) * 128],
                                t_xv[:, b * 128 : (b + 1) * 128], op=OP.add)
                        t_T = wk3.tile([128, 512], F32, tag="ttile")
                        nc.scalar.activation(t_T[:, : w * 128], t_Z[:, : w * 128],
                                             AF.Tanh, bias=t_bvbp[:, 0:1])
                        for j in range(w):
                            nc.tensor.matmul(
                                att_ps[:, filled + j : filled + j + 1],
                                t_T[:, j * 128 : (j + 1) * 128],
                                t_th[:], start=True, stop=True)
                        filled += w
                    nc.vector.tensor_copy(t_att[:, done : done + cw],
                                          att_ps[:, :cw])
                    done += cw
                # --- stats + soft/hard for this group
                t_yg = wk.tile([128, 640], F32, tag="yg")
                nc.vector.tensor_tensor(t_yg[:, :gcols], t_att[:, :gcols],
                                        t_gatt[:, gc0 : gc0 + gcols], op=OP.add)
                t_exg = wk.tile([128, 640], F32, tag="exg")
                nc.scalar.activation(t_exg[:, :gcols], t_yg[:, :gcols], AF.Exp)
                t_dn = st.tile([128, NBLK], F32, tag="den")
                t_mx = st.tile([128, NBLK], F32, tag="mx")
                nc.vector.tensor_reduce(
                    t_dn[:, g0:g1],
                    t_exg[:, :gcols].rearrange("p (n l) -> p n l", l=L),
                    axis=mybir.AxisListType.X, op=OP.add)
                nc.vector.tensor_reduce(
                    t_mx[:, g0:g1],
                    t_yg[:, :gcols].rearrange("p (n l) -> p n l", l=L),
                    axis=mybir.AxisListType.X, op=OP.max)
                t_rd = st.tile([128, NBLK], F32, tag="rden")
                nc.vector.tensor_scalar(t_rd[:, g0:g1], t_dn[:, g0:g1], 1e-30,
                                        None, op0=OP.max)
                nc.vector.reciprocal(t_rd[:, g0:g1], t_rd[:, g0:g1])
                t_sfg = wk.tile([128, 640], F32, tag="sfg")
                nc.vector.tensor_tensor(
                    t_sfg[:, :gcols].rearrange("p (n l) -> p n l", l=L),
                    t_exg[:, :gcols].rearrange("p (n l) -> p n l", l=L),
                    t_rd[:, g0:g1].to_broadcast([128, nb, L]), op=OP.mult)
                t_hdg = wk.tile([128, 640], F32, tag="hdg")
                nc.vector.tensor_tensor(
                    t_hdg[:, :gcols].rearrange("p (n l) -> p n l", l=L),
                    t_yg[:, :gcols].rearrange("p (n l) -> p n l", l=L),
                    t_mx[:, g0:g1].to_broadcast([128, nb, L]), op=OP.is_equal)
                nc.sync.dma_start(soft_g.ap()[:, gc0 : gc0 + gcols],
                                  t_sfg[:, :gcols])
                nc.vector.tensor_tensor(t_hdg[:, :gcols], t_hdg[:, :gcols],
                                        t_sfg[:, :gcols], op=OP.subtract)
                nc.vector.tensor_tensor(t_hdg[:, :gcols], t_hdg[:, :gcols],
                                        t_sfg[:, :gcols], op=OP.add)
                nc.sync.dma_start(hard_g.ap()[:, gc0 : gc0 + gcols],
                                  t_hdg[:, :gcols])
                # --- agg for this group
                t_sbf = wk.tile([128, 640], BF16, tag="sbfg")
                nc.vector.tensor_copy(t_sbf[:, :gcols], t_sfg[:, :gcols])
                for b in range(g0, g1):
                    c0 = int(colstart[b])
                    goff = c0 - gc0
                    t_xr = blk.tile([128, 128], F32, tag="xrblk")
                    nc.sync.dma_start(t_xr[:],
                                      xrows.ap()[b * 128 : (b + 1) * 128, :])
                    t_new = blk.tile([128, 128], F32, tag="newblk")
                    t_rw = blk.tile([128, LMAX * 128], BF16, tag="rwbuf")
                    nc.sync.dma_start(t_rw[:, : L * 128],
                                      praw_rows.ap()[:, c0 * 128 : (c0 + L) * 128])
                    t_gw = blk.tile([128, LMAX * 128], BF16, tag="gwbuf")
                    nc.vector.tensor_tensor(
                        t_gw[:, : L * 128].rearrange("p (l h) -> p l h", h=128),
                        t_rw[:, : L * 128].rearrange("p (l h) -> p l h", h=128),
                        t_sbf[:, goff : goff + L].to_broadcast([128, L, 128]),
                        op=OP.mult)
                    agg = psa.tile([128, 128], F32, tag="aggps")
                    for l in range(L):
                        nc.tensor.matmul(agg[:], t_ibf[:],
                                         t_gw[:, l * 128 : (l + 1) * 128],
                                         start=(l == 0), stop=(l == L - 1))
                    nc.vector.scalar_tensor_tensor(
                        t_new[:], agg[:], t_s0[:, b : b + 1], t_xr[:],
                        op0=OP.mult, op1=OP.add)
                    nc.sync.dma_start(newr.ap()[b * 128 : (b + 1) * 128, :],
                                      t_new[:])

    nc.finalize()
    return nc


# ---------------------------------------------------------------- unshard
def _unshard(results, meta, E):
    ncol = meta["ncol"]
    hard_mask = np.zeros((NV, 1), np.float32)
    soft_mask = np.zeros((NV, 1), np.float32)
    hard_att = np.zeros((E, 1), np.float32)
    soft_att = np.zeros((E, 1), np.float32)
    new_voxel = np.zeros((NV, H), np.float32)
    for c in range(N_CORES):
        r = results[c]
        lay = meta["lays"][c]
        v0 = c * NVS
        order = lay["order"]
        se = lay["slot_edge"]
        real = se >= 0
        soft_att[se[real], 0] = r["soft_g"][real]
        hard_att[se[real], 0] = r["hard_g"][real]
        # seg-order -> voxel order
        s0 = r["s0_pk"].T.reshape(-1)       # index j = b*128+p
        hm = r["hm_pk"].T.reshape(-1)
        nr = r["newr"]                      # [6272, 128] rows j
        sel = order < NVS
        vdst = v0 + order[sel]
        soft_mask[vdst, 0] = s0[sel]
        hard_mask[vdst, 0] = hm[sel]
        new_voxel[vdst] = nr[sel]
    return hard_mask, soft_mask, hard_att, soft_att, new_voxel


_CACHE = {}


def kernel(**inputs):
    _setup_env()
    from concourse.bass_utils import run_bass_kernel_spmd

    in_maps, meta = _prep(inputs)
    key = (meta["ncol"], tuple(meta["lays"][0]["Ls"]))
    nc = _CACHE.get(key)
    if nc is None:
        lay0 = meta["lays"][0]
        nc = _build(meta["ncol"], lay0["Ls"], lay0["colstart"])
        _CACHE[key] = nc
    results = run_bass_kernel_spmd(
        nc, in_maps, list(range(N_CORES)), trace=False).results
    E = len(np.asarray(inputs["cross_edge_program_index"]))
    return _unshard(results, meta, E)


# revision 15
# speedup vs baseline: 1.0309x; 1.0309x over previous
"""Trainium2 Bass kernel for nn_Attention_32220844654630 (GNN message passing).

Self-contained: kernel(**inputs) -> 5-tuple matching reference outputs.

Strategy: partition voxels across 8 cores (edges are sorted by destination
voxel, so each core gets a contiguous edge range; no collectives needed).
Per core, segments (per-voxel edge groups) are sorted by length and packed
into 49 blocks of 128 segments; each block is padded to its max segment
length L_b, giving a slot grid [128 seg-partitions x NCOL columns] with only
~4% padding.  Per-edge math runs in H-on-partitions layout:
  Z = Wv.T@X[v] (block-broadcast matmul) + Wp.T@P[p] (matmul over host-staged
      gathered raw P columns) accumulated in PSUM,
  T = tanh(Z + bv + bp) on ACT straight from PSUM,
  att = theta.T @ T per 128-slot column on PE.
Segment softmax stats are uniform blocked reduces on DVE; the weighted
scatter-sum uses host-staged P rows (bf16), a broadcast-multiply by soft_att,
and identity-matmul PSUM accumulation per block.  Host only reshards /
gathers / transposes raw inputs; all arithmetic is on device.
"""
import sys
import types
import numpy as np

N_CORES = 8
H = 128
NP_N = 1000
NV = 50000
NVS = NV // N_CORES        # 6250
NVS_PAD = 6272             # 49 * 128
NBLK = NVS_PAD // 128      # 49
GATT_PAD = np.float32(-60000.0)


# ---------------------------------------------------------------- axon hooks
def _setup_env():
    try:
        import antenv
    except ImportError:
        return
    if "antenv.axon_hooks" not in sys.modules:
        hooks = types.ModuleType("antenv.axon_hooks")
        _h = [None]
        hooks.set_axon_ntff_profile_hook = lambda h: _h.__setitem__(0, h)
        hooks.get_axon_ntff_profile_hook = lambda: _h[0]
        sys.modules["antenv.axon_hooks"] = hooks
        antenv.axon_hooks = hooks
        try:
            from trn_agent_boot.trn_boot import _ntff_profile_via_ctypes
            hook = _ntff_profile_via_ctypes("/opt/axon/libaxon_pjrt.so")
            if hook is not None:
                hooks.set_axon_ntff_profile_hook(hook)
        except Exception:
            pass
    import concourse.bass_utils as bu
    bu.upload_artifacts = lambda tmpdir: f"file://{tmpdir}"


# ---------------------------------------------------------------- host prep
def _core_layout(vidx, e0, e1, v0, Ls_force=None):
    vloc = (vidx[e0:e1] - v0).astype(np.int64)
    counts = np.bincount(vloc, minlength=NVS_PAD)
    offs = np.concatenate([[0], np.cumsum(counts)])
    order = np.argsort(counts, kind="stable")
    Ls = np.zeros(NBLK, dtype=np.int64)
    for b in range(NBLK):
        Ls[b] = counts[order[b * 128 : (b + 1) * 128]].max()
    if Ls_force is not None:
        assert (Ls_force >= Ls).all()
        Ls = Ls_force.copy()
    colstart = np.concatenate([[0], np.cumsum(Ls)])
    ncol = int(Ls.sum())
    slot_edge = np.full((128, ncol), -1, dtype=np.int64)
    for b in range(NBLK):
        segs = order[b * 128 : (b + 1) * 128]
        c0 = int(colstart[b])
        for p in range(128):
            ln = int(counts[segs[p]])
            if ln:
                slot_edge[p, c0 : c0 + ln] = e0 + offs[segs[p]] + np.arange(ln)
    return dict(Ec=e1 - e0, ncol=ncol, Ls=Ls, colstart=colstart, order=order,
                counts=counts, slot_edge=slot_edge)


def _prep(inputs):
    pidx = np.asarray(inputs["cross_edge_program_index"]).astype(np.int64)
    vidx = np.asarray(inputs["cross_edge_voxel_index"]).astype(np.int64)
    X = np.asarray(inputs["voxel_feature"], dtype=np.float32)
    P = np.asarray(inputs["program_graph_feature"], dtype=np.float32)
    g_att = np.asarray(inputs["g_att"], dtype=np.float32)
    g_mask = np.asarray(inputs["g_mask"], dtype=np.float32)

    bounds = [int(np.searchsorted(vidx, c * NVS, side="left"))
              for c in range(N_CORES)] + [len(vidx)]
    lays = [_core_layout(vidx, bounds[c], bounds[c + 1], c * NVS)
            for c in range(N_CORES)]
    # SPMD: one program for all cores -> common per-block lengths
    Ls_common = np.max([l["Ls"] for l in lays], axis=0)
    lays = [_core_layout(vidx, bounds[c], bounds[c + 1], c * NVS,
                         Ls_force=Ls_common)
            for c in range(N_CORES)]
    ncol = int(Ls_common.sum())

    itile = np.tile(np.eye(128, dtype=np.float32), (1, 4))          # [128,512]
    i128bf = np.eye(128, dtype=np.float32)

    in_maps = []
    for c, lay in enumerate(lays):
        v0 = c * NVS
        order = lay["order"]
        se = lay["slot_edge"]
        pad = se < 0
        e_ids = np.where(pad, 0, se)
        p_slot = np.where(pad, 0, pidx[e_ids]).astype(np.int64)      # [128,ncol]
        gatt = np.where(pad, GATT_PAD, g_att[e_ids].astype(np.float32))

        Xs = np.zeros((NVS_PAD, H), dtype=np.float32)
        Xs[:NVS] = X[v0 : v0 + NVS]
        Xseg = Xs[order]                                             # seg-order rows
        gmp = np.zeros((NVS_PAD, 2), dtype=np.float32)
        gmp[:NVS] = g_mask[v0 : v0 + NVS]
        gms = gmp[order]                                             # seg-order

        # P gathered per slot. prawT: [128 H, nslot] fp16, slot j = col*128+p
        Pg = P[p_slot]                                               # [128,ncol,128]
        prawT = np.ascontiguousarray(
            Pg.transpose(2, 1, 0).reshape(H, ncol * 128)).astype(np.float16)
        praw_rows = np.ascontiguousarray(
            Pg.reshape(128, ncol * 128))
        import ml_dtypes
        praw_rows = praw_rows.astype(ml_dtypes.bfloat16)

        in_maps.append(dict(
            xt=np.ascontiguousarray(Xseg.T),                         # [128,6272]
            xrows=np.ascontiguousarray(Xseg),                        # [6272,128]
            prawT=prawT,                                             # [128,nslot]
            praw_rows=praw_rows,                                     # [128,nslot]
            gatt=np.ascontiguousarray(gatt),                         # [128,ncol]
            gm0=np.ascontiguousarray(gms[:, 0].reshape(NBLK, 128).T),  # [128,49]
            gm1=np.ascontiguousarray(gms[:, 1].reshape(NBLK, 128).T),
            Wv=np.asarray(inputs["Wv"], np.float32),
            Wp=np.asarray(inputs["Wp"], np.float32),
            W1=np.asarray(inputs["W1"], np.float32),
            W2=np.asarray(inputs["W2"], np.float32),                 # [64,2]
            b2r=np.asarray(inputs["b2"], np.float32).reshape(1, 2),
            b1c=np.asarray(inputs["b1"], np.float32).reshape(64, 1),
            bvc=np.asarray(inputs["bv"], np.float32).reshape(128, 1),
            bpc=np.asarray(inputs["bp"], np.float32).reshape(128, 1),
            theta=np.asarray(inputs["theta"], np.float32).reshape(128, 1),
            itile=itile,
            i128bf=i128bf.astype(ml_dtypes.bfloat16),
        ))
    return in_maps, dict(lays=lays, bounds=bounds, ncol=ncol)


# ---------------------------------------------------------------- device
def _build(ncol, Ls, colstart):
    LMAX = int(max(Ls))
    import concourse.bass as bass
    import concourse.bacc as bacc
    import concourse.mybir as mybir
    from concourse.tile import TileContext

    dt = mybir.dt
    F32, F16, BF16 = dt.float32, dt.float16, dt.bfloat16
    AF = mybir.ActivationFunctionType
    OP = mybir.AluOpType
    nslot = ncol * 128

    nc = bacc.Bacc(None, target_bir_lowering=False, debug=False)
    P_ = {}
    def dram(name, shape, dty, out=False):
        P_[name] = nc.declare_dram_parameter(name, shape, dty, isOutput=out)
        return P_[name]

    xt = dram("xt", [128, NVS_PAD], F32)
    xrows = dram("xrows", [NVS_PAD, H], F32)
    prawT = dram("prawT", [128, nslot], F16)
    praw_rows = dram("praw_rows", [128, nslot], BF16)
    gatt = dram("gatt", [128, ncol], F32)
    gm0 = dram("gm0", [128, NBLK], F32)
    gm1 = dram("gm1", [128, NBLK], F32)
    Wv = dram("Wv", [128, 128], F32)
    Wp = dram("Wp", [128, 128], F32)
    W1 = dram("W1", [128, 64], F32)
    W2 = dram("W2", [64, 2], F32)
    b2r = dram("b2r", [1, 2], F32)
    b1c = dram("b1c", [64, 1], F32)
    bvc = dram("bvc", [128, 1], F32)
    bpc = dram("bpc", [128, 1], F32)
    theta = dram("theta", [128, 1], F32)
    itile = dram("itile", [128, 512], F32)
    i128bf = dram("i128bf", [128, 128], BF16)

    soft_g = dram("soft_g", [128, ncol], F32, out=True)
    hard_g = dram("hard_g", [128, ncol], F32, out=True)
    s0_pk = dram("s0_pk", [128, NBLK], F32, out=True)
    hm_pk = dram("hm_pk", [128, NBLK], F32, out=True)
    newr = dram("newr", [NVS_PAD, H], F32, out=True)

    ddram = nc.dram_tensor("ddram", [1, NVS_PAD], F32)

    # distinct-L groups (consecutive blocks with equal L)
    groups = []
    b = 0
    while b < NBLK:
        e = b
        while e < NBLK and Ls[e] == Ls[b]:
            e += 1
        groups.append((b, e, int(Ls[b])))
        b = e

    with TileContext(nc) as tc:
        with (
            tc.tile_pool(name="const", bufs=1) as cst,
            tc.tile_pool(name="tab", bufs=1) as tab,
            tc.tile_pool(name="st", bufs=1) as st,
            tc.tile_pool(name="wk", bufs=2) as wk,
            tc.tile_pool(name="blk", bufs=2) as blk,
            tc.tile_pool(name="ps", bufs=1, space="PSUM") as ps,
            tc.tile_pool(name="psz", bufs=3, space="PSUM") as psz,
            tc.tile_pool(name="psa", bufs=2, space="PSUM") as psa,
        ):
            # ---- load constants / tables
            t_xt = tab.tile([128, NVS_PAD], F32, tag="xt")
            nc.sync.dma_start(t_xt[:], xt.ap())
            t_Wv = cst.tile([128, 128], F32, tag="wv"); nc.sync.dma_start(t_Wv[:], Wv.ap())
            t_Wp = cst.tile([128, 128], F32, tag="wp"); nc.sync.dma_start(t_Wp[:], Wp.ap())
            t_W1 = cst.tile([128, 64], F32, tag="w1"); nc.sync.dma_start(t_W1[:], W1.ap())
            t_W2 = cst.tile([64, 2], F32, tag="w2"); nc.sync.dma_start(t_W2[:], W2.ap())
            t_b2r = cst.tile([1, 2], F32, tag="b2r"); nc.sync.dma_start(t_b2r[:], b2r.ap())
            t_b1c = cst.tile([64, 1], F32, tag="b1c"); nc.sync.dma_start(t_b1c[:], b1c.ap())
            t_bvc = cst.tile([128, 1], F32, tag="bvc"); nc.sync.dma_start(t_bvc[:], bvc.ap())
            t_bpc = cst.tile([128, 1], F32, tag="bpc"); nc.sync.dma_start(t_bpc[:], bpc.ap())
            t_th = cst.tile([128, 1], F32, tag="th"); nc.sync.dma_start(t_th[:], theta.ap())
            t_ibf = cst.tile([128, 128], BF16, tag="ibf"); nc.sync.dma_start(t_ibf[:], i128bf.ap())
            t_gatt = tab.tile([128, ncol], F32, tag="gatt")
            nc.sync.dma_start(t_gatt[:], gatt.ap())
            t_gm0 = cst.tile([128, NBLK], F32, tag="gm0"); nc.sync.dma_start(t_gm0[:], gm0.ap())
            t_gm1 = cst.tile([128, NBLK], F32, tag="gm1"); nc.sync.dma_start(t_gm1[:], gm1.ap())

            t_Wp16 = cst.tile([128, 128], F16, tag="wp16")
            nc.vector.tensor_copy(t_Wp16[:], t_Wp[:])
            t_bvbp = cst.tile([128, 1], F32, tag="bvbp")
            nc.vector.tensor_tensor(t_bvbp[:], t_bvc[:], t_bpc[:], op=OP.add)
            t_W2d = cst.tile([64, 1], F32, tag="w2d")
            nc.vector.tensor_tensor(t_W2d[:], t_W2[:, 1:2], t_W2[:, 0:1], op=OP.subtract)
            t_b2d = cst.tile([1, 1], F32, tag="b2d")
            nc.vector.tensor_tensor(t_b2d[:], t_b2r[:, 1:2], t_b2r[:, 0:1], op=OP.subtract)

            # ---- XvT = Wv.T @ xt  -> [128 H, 6272 seg]
            t_xv = tab.tile([128, NVS_PAD], F32, tag="xvt")
            for n in range(0, NVS_PAD, 512):
                w0 = min(512, NVS_PAD - n)
                p0 = ps.tile([128, 512], F32, tag="ps_small")
                nc.tensor.matmul(p0[:, :w0], t_Wv[:], t_xt[:, n : n + w0],
                                 start=True, stop=True)
                nc.vector.tensor_copy(t_xv[:, n : n + w0], p0[:, :w0])

            # ---- mask MLP: h1T = W1.T @ xt + b1 ; d = W2d.T @ h1T + b2d
            t_h1 = tab.tile([64, NVS_PAD], F32, tag="h1t")
            for n in range(0, NVS_PAD, 512):
                w = min(512, NVS_PAD - n)
                p0 = ps.tile([128, 512], F32, tag="ps_small")
                nc.tensor.matmul(p0[:64, :w], t_W1[:], t_xt[:, n : n + w],
                                 start=True, stop=True)
                nc.vector.tensor_scalar(t_h1[:, n : n + w], p0[:64, :w],
                                        t_b1c[:, 0:1], None, op0=OP.add)
            t_d = st.tile([1, NVS_PAD], F32, tag="dtile")
            for n in range(0, NVS_PAD, 512):
                w = min(512, NVS_PAD - n)
                p0 = ps.tile([128, 512], F32, tag="ps_small")
                nc.tensor.matmul(p0[:1, :w], t_W2d[:], t_h1[:, n : n + w],
                                 start=True, stop=True)
                nc.vector.tensor_scalar(t_d[:, n : n + w], p0[:1, :w],
                                        t_b2d[:, 0:1], None, op0=OP.add)
            nc.sync.dma_start(ddram[:], t_d[:])
            t_dpk = st.tile([128, NBLK], F32, tag="dpk")
            nc.sync.dma_start(
                t_dpk[:], ddram.ap().rearrange("a (b p) -> p (a b)", p=128))
            t_dg = st.tile([128, NBLK], F32, tag="dg")
            nc.vector.tensor_tensor(t_dg[:], t_dpk[:], t_gm1[:], op=OP.add)
            nc.vector.tensor_tensor(t_dg[:], t_dg[:], t_gm0[:], op=OP.subtract)
            t_s0 = st.tile([128, NBLK], F32, tag="s0")
            nc.scalar.activation(t_s0[:], t_dg[:], AF.Sigmoid, scale=-1.0)
            t_h0 = st.tile([128, NBLK], F32, tag="h0")
            nc.vector.tensor_scalar(t_h0[:], t_dg[:], 0.0, None, op0=OP.is_le)
            t_hm = st.tile([128, NBLK], F32, tag="hm")
            nc.vector.tensor_tensor(t_hm[:], t_h0[:], t_s0[:], op=OP.subtract)
            nc.vector.tensor_tensor(t_hm[:], t_hm[:], t_s0[:], op=OP.add)
            nc.sync.dma_start(s0_pk.ap(), t_s0[:])
            nc.sync.dma_start(hm_pk.ap(), t_hm[:])

            # ---- pipelined per-L-group: att -> stats -> soft/hard -> agg
            for (g0, g1, L) in groups:
                if L == 0:
                    for b in range(g0, g1):
                        t_xr = blk.tile([128, 128], F32, tag="xrblk")
                        nc.sync.dma_start(t_xr[:],
                                          xrows.ap()[b * 128 : (b + 1) * 128, :])
                        t_new = blk.tile([128, 128], F32, tag="newblk")
                        nc.vector.tensor_copy(t_new[:], t_xr[:])
                        nc.sync.dma_start(newr.ap()[b * 128 : (b + 1) * 128, :],
                                          t_new[:])
                    continue
                nb = g1 - g0
                gc0 = int(colstart[g0])
                gcols = nb * L
                # --- att for this group
                t_att = wk.tile([128, 640], F32, tag="attg")
                done = 0
                while done < gcols:
                    cw = min(512, gcols - done)
                    att_ps = psa.tile([128, 512], F32, tag="attps")
                    filled = 0
                    while filled < cw:
                        b = g0 + (done + filled) // L
                        l = (done + filled) % L
                        w = min(4, L - l, cw - filled)
                        c0 = int(colstart[b])
                        if l == 0:
                            t_pr = wk.tile([128, LMAX * 128], F16, tag="prbuf")
                            nc.sync.dma_start(
                                t_pr[:, : L * 128],
                                prawT.ap()[:, c0 * 128 : (c0 + L) * 128])
                        zp = psz.tile([128, 512], F32, tag="zps")
                        nc.tensor.matmul(zp[:, : w * 128], t_Wp16[:],
                                         t_pr[:, l * 128 : (l + w) * 128],
                                         start=True, stop=True)
                        t_Z = wk.tile([128, 512], F32, tag="ztile")
                        for j in range(w):
                            nc.vector.tensor_tensor(
                                t_Z[:, j * 128 : (j + 1) * 128],
                                zp[:, j * 128 : (j + 1) * 128],
                                t_xv[:, b * 128 : (b + 1) * 128], op=OP.add)
                        t_T = wk.tile([128, 512], F32, tag="ttile")
                        nc.scalar.activation(t_T[:, : w * 128], t_Z[:, : w * 128],
                                             AF.Tanh, bias=t_bvbp[:, 0:1])
                        for j in range(w):
                            nc.tensor.matmul(
                                att_ps[:, filled + j : filled + j + 1],
                                t_T[:, j * 128 : (j + 1) * 128],
                                t_th[:], start=True, stop=True)
                        filled += w
                    nc.vector.tensor_copy(t_att[:, done : done + cw],
                                          att_ps[:, :cw])
                    done += cw
                # --- stats + soft/hard for this group
                t_yg = wk.tile([128, 640], F32, tag="yg")
                nc.vector.tensor_tensor(t_yg[:, :gcols], t_att[:, :gcols],
                                        t_gatt[:, gc0 : gc0 + gcols], op=OP.add)
                t_exg = wk.tile([128, 640], F32, tag="exg")
                nc.scalar.activation(t_exg[:, :gcols], t_yg[:, :gcols], AF.Exp)
                t_dn = st.tile([128, NBLK], F32, tag="den")
                t_mx = st.tile([128, NBLK], F32, tag="mx")
                nc.vector.tensor_reduce(
                    t_dn[:, g0:g1],
                    t_exg[:, :gcols].rearrange("p (n l) -> p n l", l=L),
                    axis=mybir.AxisListType.X, op=OP.add)
                nc.vector.tensor_reduce(
                    t_mx[:, g0:g1],
                    t_yg[:, :gcols].rearrange("p (n l) -> p n l", l=L),
                    axis=mybir.AxisListType.X, op=OP.max)
                t_rd = st.tile([128, NBLK], F32, tag="rden")
                nc.vector.tensor_scalar(t_rd[:, g0:g1], t_dn[:, g0:g1], 1e-30,
                                        None, op0=OP.max)
                nc.vector.reciprocal(t_rd[:, g0:g1], t_rd[:, g0:g1])
                t_sfg = wk.tile([128, 640], F32, tag="sfg")
                nc.vector.tensor_tensor(
                    t_sfg[:, :gcols].rearrange("p (n l) -> p n l", l=L),
                    t_exg[:, :gcols].rearrange("p (n l) -> p n l", l=L),
                    t_rd[:, g0:g1].to_broadcast([128, nb, L]), op=OP.mult)
                t_hdg = wk.tile([128, 640], F32, tag="hdg")
                nc.vector.tensor_tensor(
                    t_hdg[:, :gcols].rearrange("p (n l) -> p n l", l=L),
                    t_yg[:, :gcols].rearrange("p (n l) -> p n l", l=L),
                    t_mx[:, g0:g1].to_broadcast([128, nb, L]), op=OP.is_equal)
                nc.sync.dma_start(soft_g.ap()[:, gc0 : gc0 + gcols],
                                  t_sfg[:, :gcols])
                nc.vector.tensor_tensor(t_hdg[:, :gcols], t_hdg[:, :gcols],
                                        t_sfg[:, :gcols], op=OP.subtract)
                nc.vector.tensor_tensor(t_hdg[:, :gcols], t_hdg[:, :gcols],
                                        t_sfg[:, :gcols], op=OP.add)
                nc.sync.dma_start(hard_g.ap()[:, gc0 : gc0 + gcols],
                                  t_hdg[:, :gcols])
                # --- agg for this group
                t_sbf = wk.tile([128, 640], BF16, tag="sbfg")
                nc.vector.tensor_copy(t_sbf[:, :gcols], t_sfg[:, :gcols])
                for b in range(g0, g1):
                    c0 = int(colstart[b])
                    goff = c0 - gc0
                    t_xr = blk.tile([128, 128], F32, tag="xrblk")
                    nc.sync.dma_start(t_xr[:],
                                      xrows.ap()[b * 128 : (b + 1) * 128, :])
                    t_new = blk.tile([128, 128], F32, tag="newblk")
                    t_rw = blk.tile([128, LMAX * 128], BF16, tag="rwbuf")
                    nc.sync.dma_start(t_rw[:, : L * 128],
                                      praw_rows.ap()[:, c0 * 128 : (c0 + L) * 128])
                    t_gw = blk.tile([128, LMAX * 128], BF16, tag="gwbuf")
                    nc.vector.tensor_tensor(
                        t_gw[:, : L * 128].rearrange("p (l h) -> p l h", h=128),
                        t_rw[:, : L * 128].rearrange("p (l h) -> p l h", h=128),
                        t_sbf[:, goff : goff + L].to_broadcast([128, L, 128]),
                        op=OP.mult)
                    agg = psa.tile([128, 128], F32, tag="aggps")
                    for l in range(L):
                        nc.tensor.matmul(agg[:], t_ibf[:],
                                         t_gw[:, l * 128 : (l + 1) * 128],
                                         start=(l == 0), stop=(l == L - 1))
                    nc.vector.scalar_tensor_tensor(
                        t_new[:], agg[:], t_s0[:, b : b + 1], t_xr[:],
                        op0=OP.mult, op1=OP.add)
                    nc.sync.dma_start(newr.ap()[b * 128 : (b + 1) * 128, :],
                                      t_new[:])

    nc.finalize()
    return nc


# ---------------------------------------------------------------- unshard
def _unshard(results, meta, E):
    ncol = meta["ncol"]
    hard_mask = np.zeros((NV, 1), np.float32)
    soft_mask = np.zeros((NV, 1), np.float32)
    hard_att = np.zeros((E, 1), np.float32)
    soft_att = np.zeros((E, 1), np.float32)
    new_voxel = np.zeros((NV, H), np.float32)
    for c in range(N_CORES):
        r = results[c]
        lay = meta["lays"][c]
        v0 = c * NVS
        order = lay["order"]
        se = lay["slot_edge"]
        real = se >= 0
        soft_att[se[real], 0] = r["soft_g"][real]
        hard_att[se[real], 0] = r["hard_g"][real]
        # seg-order -> voxel order
        s0 = r["s0_pk"].T.reshape(-1)       # index j = b*128+p
        hm = r["hm_pk"].T.reshape(-1)
        nr = r["newr"]                      # [6272, 128] rows j
        sel = order < NVS
        vdst = v0 + order[sel]
        soft_mask[vdst, 0] = s0[sel]
        hard_mask[vdst, 0] = hm[sel]
        new_voxel[vdst] = nr[sel]
    return hard_mask, soft_mask, hard_att, soft_att, new_voxel


_CACHE = {}


def kernel(**inputs):
    _setup_env()
    from concourse.bass_utils import run_bass_kernel_spmd

    in_maps, meta = _prep(inputs)
    key = (meta["ncol"], tuple(meta["lays"][0]["Ls"]))
    nc = _CACHE.get(key)
    if nc is None:
        lay0 = meta["lays"][0]
        nc = _build(meta["ncol"], lay0["Ls"], lay0["colstart"])
        _CACHE[key] = nc
    results = run_bass_kernel_spmd(
        nc, in_maps, list(range(N_CORES)), trace=False).results
    E = len(np.asarray(inputs["cross_edge_program_index"]))
    return _unshard(results, meta, E)
